# revision 2
# baseline (speedup 1.0000x reference)
# BinsCombinerLayer Trainium2 kernel.
#
#   out[b] = (1/NUM_BINS) * sum_{n,s} inputs[b,n,s] * centroids[n,s]
#
# Pure data parallel over 8 NeuronCores: each core takes B/8 = 4096 examples.
# The dot product runs on the PE array in bf16 (the 2e-2 tolerance leaves
# orders of magnitude of slack), which halves the HBM traffic vs f32 --
# the binding roofline for this kernel -- and frees the DVE entirely.
#
# Host-side prep per core: x slice [4096, 2048] f32 is cast to bf16 and
# transposed to xT [2048, 4096] (d-major) so the PE can contract over the
# partition axis: for each 128-row d-chunk k, matmul(psum[1, N], lhsT =
# cbT[:, k] [128, 1], rhs = xT_k [128, N]) accumulates the per-example
# partial dots over the 16 chunks in PSUM.  Centroids are pre-scaled by
# 1/NUM_BINS and transposed to [128, 16] on host (tiny).
import numpy as np

import concourse.bacc as bacc
import concourse.mybir as mybir
import concourse.tile as tile
from concourse.bass_utils import run_bass_kernel_spmd

N_CORES = 8
B, NUM_BINS, BIN_SIZE = 32768, 16, 128
D = NUM_BINS * BIN_SIZE      # 2048 f32 per example
P = 128                      # SBUF partitions
BC = B // N_CORES            # 4096 examples per core
K = D // P                   # 16 d-chunks of 128
F32 = mybir.dt.float32
BF16 = mybir.dt.bfloat16

_CACHED = None


def _build_program(repeat=1, qw=1024, nblk=512, bufs=12):
    """qw: examples per DMA tile; nblk: examples per PSUM accumulation group."""
    nc = bacc.Bacc("TRN2", target_bir_lowering=False, debug=False)
    x = nc.dram_tensor("x", [D, BC], BF16, kind="ExternalInput").ap()
    cb = nc.dram_tensor("cb", [P, K], BF16, kind="ExternalInput").ap()
    out = nc.dram_tensor("out", [1, BC], F32, kind="ExternalOutput").ap()

    nq = BC // qw
    with tile.TileContext(nc) as tc:
        with (
            tc.tile_pool(name="xin", bufs=bufs) as xpool,
            tc.tile_pool(name="misc", bufs=1) as misc,
            tc.tile_pool(name="ps", bufs=4, space="PSUM") as pspool,
        ):
            cbt = misc.tile([P, K], BF16)
            nc.sync.dma_start(out=cbt[:], in_=cb[:])
            collect = misc.tile([1, BC], F32)

            for _ in range(repeat):
                for q in range(nq):
                    xts = []
                    for k in range(K):
                        xt = xpool.tile([P, qw], BF16, tag="xt")
                        nc.sync.dma_start(
                            out=xt[:],
                            in_=x[k * P : (k + 1) * P, q * qw : (q + 1) * qw],
                        )
                        xts.append(xt)
                    for blk in range(qw // nblk):
                        ps = pspool.tile([1, nblk], F32, tag="ps")
                        lo = blk * nblk
                        for k in range(K):
                            nc.tensor.matmul(
                                ps[:],
                                cbt[:, k : k + 1],
                                xts[k][:, lo : lo + nblk],
                                start=(k == 0),
                                stop=(k == K - 1),
                            )
                        nc.scalar.copy(
                            collect[:, q * qw + lo : q * qw + lo + nblk], ps[:]
                        )

                nc.sync.dma_start(out=out[:], in_=collect[:])

    nc.compile()
    return nc


def _get_program():
    global _CACHED
    if _CACHED is None:
        _CACHED = _build_program()
    return _CACHED


def _prep_inputs(inputs, centroids):
    import ml_dtypes

    bf16 = ml_dtypes.bfloat16
    x = np.asarray(inputs, dtype=np.float32).reshape(N_CORES, BC, D)
    # cast + transpose to per-core [D, BC] bf16 (d-major, examples contiguous)
    xT = np.ascontiguousarray(x.transpose(0, 2, 1)).astype(bf16)
    c = np.asarray(centroids, dtype=np.float32).reshape(D) / NUM_BINS
    # cbT[p, k] = c[k*128 + p]
    cbT = np.ascontiguousarray(c.astype(bf16).reshape(K, P).T)
    return xT, cbT


def run(inputs, centroids, **spmd_kwargs):
    """Run the kernel; returns (full_output, BassKernelResults)."""
    nc = _get_program()
    xT, cbT = _prep_inputs(inputs, centroids)
    in_maps = [{"x": xT[i], "cb": cbT} for i in range(N_CORES)]
    try:
        res = run_bass_kernel_spmd(
            nc, in_maps, list(range(N_CORES)), **spmd_kwargs
        )
    except Exception:
        # transient NRT_EXEC_UNIT_UNRECOVERABLE wedges recover on retry
        res = run_bass_kernel_spmd(
            nc, in_maps, list(range(N_CORES)), **spmd_kwargs
        )
    full = np.concatenate([r["out"].reshape(BC) for r in res.results])
    return full.astype(np.float32, copy=False), res


def kernel(inputs, centroids):
    full, _ = run(inputs, centroids)
    return full
